# revision 12
# baseline (speedup 1.0000x reference)
"""Multi-head causal self-attention (B=2, S=2048, E=1024, H=16, D=64) on 8 TRN2
NeuronCores.

Sharding: core c owns batch b = c//4 and head-group g = c%4 (4 heads each).
Per core, transpose-free layout: QT/KT [d_local=256, S] (d on partitions),
V [S, d_local] (t on partitions), scoresT [t, s] blocks via lhsT=KT-block,
softmax is unnormalized exp (scores ~N(0,1) in f32), sums come free from a
ones-augmented V ([V|1] -> M=65 AV matmul, row 64 = column sums), and the
output projection is row-parallel: each core computes a partial [S, E]; the
host sums the 4 partials per batch and adds bp.

v2 (software pipeline): QKV/V projection chains for s-block i+1 and the
previous block's output-projection units are DRIPPED one per j-iteration into
s-block i's attention loop, so the PE never idles at s-block boundaries (the
baseline went HAM-cold there 4x, ~26us penalty) and ScalarE exp streams
back-to-back from ~12us onward. Attention runs pair-major (head pair p's full
j-loop, then p+1) halving live AV PSUM tiles; PSUM pools: scores 2x[128,1024]
(4 banks), AV 2x[65,512] (2), shared acc 2x[128,512] (2). All PSUM->SBUF
consumers sit on VectorE (bias-adds, out copies, normalize) so the busy
ScalarE FIFO never gates PSUM recycling; softmax recip comes from DVE
reciprocal_approx_fast directly on the AV ones-row (no ACT rsum copies).

Weights/activations cast to bf16 host-side (matmul inputs); accumulation f32
in PSUM; softmax exp in f32.
"""

import numpy as np
import ml_dtypes

import concourse.bass as bass
import concourse.tile as tile
from concourse import bacc, mybir
from concourse import bass_utils

B, S, E, H, D = 2, 2048, 1024, 16, 64
NCORES = 8
HPC = 4                 # heads per core
EL = HPC * D            # 256 local channels
SBW = 512               # s-block width
NSB = S // SBW          # 4
TBW = 128               # t-block width
NTB = S // TBW          # 16
NEB = E // 128          # 8 e-blocks
SCALE = 1.0 / np.sqrt(D)

F32 = mybir.dt.float32
BF16 = mybir.dt.bfloat16

_BUILT = None


def _emit(tc, nc, d):
    Exp = mybir.ActivationFunctionType.Exp

    with (
        tc.tile_pool(name="const", bufs=1) as cst,
        tc.tile_pool(name="big", bufs=1) as big,
        tc.tile_pool(name="ptp", bufs=4) as ptp,
        tc.tile_pool(name="rsp", bufs=4) as rsp,
        tc.tile_pool(name="outp", bufs=4) as outp,
        tc.tile_pool(name="scp", bufs=2, space="PSUM") as scp,
        tc.tile_pool(name="avp", bufs=2, space="PSUM") as avp,
        tc.tile_pool(name="accp", bufs=2, space="PSUM") as accp,
    ):
        # ---- load inputs (order = arrival priority) ----
        wq = big.tile([128, NEB * EL], BF16, name="wq", tag="wq")
        wq3 = wq.rearrange("p (j t c) -> p j t c", t=2, c=128)
        wqd3 = d["wq"].rearrange("p (j t c) -> p j t c", t=2, c=128)
        nc.sync.dma_start(wq3[:, :, 0:1], wqd3[:, :, 0:1])
        bq = cst.tile([128, 2], F32, name="bq", tag="bq")
        nc.sync.dma_start(bq[:], d["bq"][:])
        # x^T e-block tiles, DMA'd in s-block chunks so compute starts early
        xt = [big.tile([128, S], BF16, name=f"xt{j}", tag=f"xt{j}")
              for j in range(NEB)]
        for j in range(NEB):
            nc.sync.dma_start(
                xt[j][:, 0:SBW], d["xt"][:, j * S: j * S + SBW]
            )
        nc.sync.dma_start(wq3[:, :, 1:2], wqd3[:, :, 1:2])
        wk = big.tile([128, NEB * EL], BF16, name="wk", tag="wk")
        nc.sync.dma_start(wk[:], d["wk"][:])
        bk = cst.tile([128, 2], F32, name="bk", tag="bk")
        nc.sync.dma_start(bk[:], d["bk"][:])
        wv = big.tile([128, NEB * EL], BF16, name="wv", tag="wv")
        nc.sync.dma_start(wv[:], d["wv"][:])
        bv = cst.tile([128, EL], F32, name="bv", tag="bv")
        nc.sync.dma_start(bv[:], d["bv"][:])
        for i in range(1, NSB):
            for j in range(NEB):
                nc.sync.dma_start(
                    xt[j][:, i * SBW:(i + 1) * SBW],
                    d["xt"][:, j * S + i * SBW: j * S + (i + 1) * SBW],
                )
        wp = big.tile([128, 2 * E], BF16, name="wp", tag="wp")
        nc.sync.dma_start(wp[:], d["wp"][:])
        tri = cst.tile([128, 128], BF16, name="tri", tag="tri")
        nc.sync.dma_start(tri[:], d["tri"][:])
        onesb = cst.tile([128, 64], BF16, name="onesb", tag="onesb")
        nc.vector.memset(onesb[:], 1.0)


        # V tiles [128, 4*65]: head h at cols 65h..65h+64, ones col at 65h+64
        vt = []
        for j in range(NTB):
            t = big.tile([128, HPC * 65], BF16, name=f"vt{j}", tag=f"vt{j}")
            nc.vector.memset(
                t.rearrange("p (h c) -> p h c", c=65)[:, :, 64:65], 1.0
            )
            vt.append(t)

        qt = [big.tile([128, S], BF16, name=f"qt{k}", tag=f"qt{k}")
              for k in range(2)]
        kt = [big.tile([128, S], BF16, name=f"kt{k}", tag=f"kt{k}")
              for k in range(2)]
        yt = [big.tile([128, S], BF16, name=f"yt{k}", tag=f"yt{k}")
              for k in range(2)]

        # ---- pipeline work units (each ~0.5-2us of PE work) ----
        def qkv_chain(i, dst, wl, bl, dt_i):
            def emit():
                ac = accp.tile([128, SBW], F32, name="qk_ac", tag="acc")
                for j in range(NEB):
                    nc.tensor.matmul(
                        ac[:],
                        wl[:, j * EL + dt_i * 128: j * EL + dt_i * 128 + 128],
                        xt[j][:, i * SBW:(i + 1) * SBW],
                        start=(j == 0),
                        stop=(j == NEB - 1),
                    )
                nc.vector.tensor_scalar_add(
                    dst[dt_i][:, i * SBW:(i + 1) * SBW], ac[:],
                    bl[:, dt_i:dt_i + 1],
                )
            return emit

        def v_chain(j16):
            def emit():
                ac = accp.tile([128, EL], F32, name="v_ac", tag="acc")
                for eb in range(NEB):
                    nc.tensor.matmul(
                        ac[:],
                        xt[eb][:, j16 * TBW:(j16 + 1) * TBW],
                        wv[:, eb * EL:(eb + 1) * EL],
                        start=(eb == 0),
                        stop=(eb == NEB - 1),
                    )
                nc.vector.tensor_add(
                    vt[j16].rearrange("p (h c) -> p h c", c=65)[:, :, 0:64],
                    ac.rearrange("p (h c) -> p h c", c=64),
                    bv.rearrange("p (h c) -> p h c", c=64),
                )
            return emit

        def a_units(i):
            u = []
            for dst, wl, bl in ((qt, wq, bq), (kt, wk, bk)):
                for dt_i in range(2):
                    u.append(qkv_chain(i, dst, wl, bl, dt_i))
            for t4 in range(4):
                u.append(v_chain(4 * i + t4))
            return u

        Copy = mybir.ActivationFunctionType.Copy

        def proj_unit(r0, nb2):
            def emit():
                pr = accp.tile([128, 512], F32, name="pr", tag="acc")
                for cb in range(2):
                    nc.tensor.matmul(
                        pr[:],
                        yt[cb][:, r0:r0 + 128],
                        wp[:, cb * E + nb2 * 512: cb * E + (nb2 + 1) * 512],
                        start=(cb == 0),
                        stop=(cb == 1),
                    )
                ot = outp.tile([128, 512], F32, name="ot", tag="ot")
                if nb2 == 0:  # split PSUM->SBUF copies across ACT and DVE
                    nc.scalar.activation(ot[:], pr[:], Copy)
                else:
                    nc.vector.tensor_copy(ot[:], pr[:])
                nc.sync.dma_start(
                    d["out"][r0:r0 + 128, nb2 * 512:(nb2 + 1) * 512], ot[:]
                )
            return emit

        def c_units(i):
            return [proj_unit(i * SBW + st * 128, nb2)
                    for st in range(4) for nb2 in range(2)]

        pending = []

        def drip(n=1):
            for _ in range(n):
                if pending:
                    pending.pop(0)()

        # ---- A(0) up front, then pipelined attention ----
        for u in a_units(0):
            u()

        for i in range(NSB):
            if i + 1 < NSB:
                pending.extend(a_units(i + 1))
            njs = 4 * i + 4
            for p in range(2):
                avs = [avp.tile([65, SBW], F32, name=f"av{hh}", tag="av")
                       for hh in range(2)]

                def av_mms(pt_, w_, j_):
                    for hh in range(2):
                        nc.tensor.matmul(
                            avs[hh][:, w_:SBW],
                            vt[j_][:, 65 * (2 * p + hh): 65 * (2 * p + hh) + 65],
                            pt_[:, hh * SBW + w_:(hh + 1) * SBW],
                            start=(j_ == 0),
                            stop=(j_ == njs - 1),
                        )

                prev = None
                for j in range(njs):
                    w = 128 * (j - 4 * i) if j >= 4 * i else 0
                    cw = SBW - w
                    sc2 = scp.tile([128, 2 * SBW], F32, name="sc2", tag="sc")
                    for hh in range(2):
                        po = 64 * hh
                        nc.tensor.matmul(
                            sc2[:, hh * SBW: hh * SBW + cw],
                            kt[p][po:po + 64, j * TBW:(j + 1) * TBW],
                            qt[p][po:po + 64, i * SBW + w: (i + 1) * SBW],
                            start=True,
                            stop=True,
                        )
                    pt_t = ptp.tile([128, 2 * SBW], BF16, name="ptile",
                                    tag="pt")
                    nc.scalar.activation(
                        pt_t.rearrange("q (g c) -> q g c", c=SBW)[:, :, w:SBW],
                        sc2.rearrange("q (g c) -> q g c", c=SBW)[:, :, 0:cw],
                        Exp,
                    )
                    if j >= 4 * i:  # diagonal: 0/1 triangular mask on PT
                        for hh in range(2):
                            zone = hh * SBW + w
                            nc.vector.tensor_mul(
                                pt_t[:, zone: zone + 128],
                                pt_t[:, zone: zone + 128],
                                tri[:],
                            )
                    if prev is not None:
                        av_mms(*prev)
                    prev = (pt_t, w, j)
                    drip(2)
                av_mms(*prev)
                # normalize pair p: yt[p][64hh+..] = av[0:64] / av[64]
                # (DVE copy-cast the ones-row to bf16 SBUF, PE-broadcast it
                #  to 64 partitions with a K=1 bf16 matmul, recip on DVE)
                for hh in range(2):
                    rsum = rsp.tile([65, SBW], BF16, name="rsum", tag="rs")
                    nc.scalar.activation(rsum[64:65, :], avs[hh][64:65, :],
                                         Copy)
                    bc = accp.tile([64, SBW], F32, name="bc", tag="acc")
                    nc.tensor.matmul(
                        bc[:], onesb[64:65, 0:64], rsum[64:65, :],
                        start=True, stop=True,
                    )
                    bcr = rsp.tile([64, SBW], F32, name="bcr", tag="bcs")
                    nc.vector.reciprocal_approx_fast(bcr[:], bc[:])
                    nc.vector.tensor_mul(
                        yt[p][64 * hh: 64 * hh + 64, i * SBW:(i + 1) * SBW],
                        avs[hh][0:64, :],
                        bcr[:],
                    )
            cu = c_units(i)
            if i < NSB - 1:
                pending.extend(cu)
            else:
                for u in cu:
                    u()


def _build():
    global _BUILT
    if _BUILT is not None:
        return _BUILT
    nc = bacc.Bacc("TRN2", target_bir_lowering=False, debug=False,
                   num_devices=NCORES)
    d = {
        "xt": nc.dram_tensor("xt", [128, NEB * S], BF16, kind="ExternalInput").ap(),
        "wq": nc.dram_tensor("wq", [128, NEB * EL], BF16, kind="ExternalInput").ap(),
        "wk": nc.dram_tensor("wk", [128, NEB * EL], BF16, kind="ExternalInput").ap(),
        "wv": nc.dram_tensor("wv", [128, NEB * EL], BF16, kind="ExternalInput").ap(),
        "wp": nc.dram_tensor("wp", [128, 2 * E], BF16, kind="ExternalInput").ap(),
        "bq": nc.dram_tensor("bq", [128, 2], F32, kind="ExternalInput").ap(),
        "bk": nc.dram_tensor("bk", [128, 2], F32, kind="ExternalInput").ap(),
        "bv": nc.dram_tensor("bv", [128, EL], F32, kind="ExternalInput").ap(),
        "tri": nc.dram_tensor("tri", [128, 128], BF16, kind="ExternalInput").ap(),
        "out": nc.dram_tensor("out", [S, E], F32, kind="ExternalOutput").ap(),
    }
    with tile.TileContext(nc) as tc:
        _emit(tc, nc, d)
    nc.compile()
    _BUILT = nc
    return _BUILT


def _blockify(a, pblk):
    """[N*pblk, M] -> [pblk, N*M] with block-column layout."""
    n = a.shape[0] // pblk
    return np.ascontiguousarray(
        a.reshape(n, pblk, a.shape[1]).transpose(1, 0, 2).reshape(pblk, -1)
    )


def _prep_core(c, x, Wq, bq, Wk, bk, Wv, bv, Wp):
    b, g = c // 4, c % 4
    lo = EL * g
    bf = ml_dtypes.bfloat16

    xT = np.ascontiguousarray(x[b].T)                        # [E, S]
    wqT = np.ascontiguousarray(Wq[lo:lo + EL, :].T) * SCALE  # [E, 256]
    wkT = np.ascontiguousarray(Wk[lo:lo + EL, :].T)
    wvT = np.ascontiguousarray(Wv[lo:lo + EL, :].T)
    wpT = np.ascontiguousarray(Wp[:, lo:lo + EL].T)          # [256, E]

    col = np.arange(128, dtype=np.int64)
    tri = np.where(col[None, :] >= np.arange(128)[:, None], 1.0, 0.0)

    return {
        "xt": _blockify(xT, 128).astype(bf),
        "wq": _blockify(wqT, 128).astype(bf),
        "wk": _blockify(wkT, 128).astype(bf),
        "wv": _blockify(wvT, 128).astype(bf),
        "wp": _blockify(wpT, 128).astype(bf),
        "bq": np.ascontiguousarray(
            (bq[lo:lo + EL] * SCALE).reshape(2, 128).T).astype(np.float32),
        "bk": np.ascontiguousarray(
            bk[lo:lo + EL].reshape(2, 128).T).astype(np.float32),
        "bv": np.ascontiguousarray(
            np.broadcast_to(bv[lo:lo + EL], (128, EL))).astype(np.float32),
        "tri": tri.astype(bf),
    }


def run(inputs, trace=False):
    """Run on hardware. Returns (out [B,S,E] f32, exec_time_ns or None)."""
    x = np.asarray(inputs["x"], np.float32)
    Wq = np.asarray(inputs["Wq"], np.float32)
    bq = np.asarray(inputs["bq"], np.float32)
    Wk = np.asarray(inputs["Wk"], np.float32)
    bk = np.asarray(inputs["bk"], np.float32)
    Wv = np.asarray(inputs["Wv"], np.float32)
    bv = np.asarray(inputs["bv"], np.float32)
    Wp = np.asarray(inputs["Wp"], np.float32)
    bp = np.asarray(inputs["bp"], np.float32)

    nc = _build()
    in_maps = [
        _prep_core(c, x, Wq, bq, Wk, bk, Wv, bv, Wp) for c in range(NCORES)
    ]
    kwargs = {}
    if trace:
        try:
            import ntff_shim
            ntff_shim.install()
        except Exception:
            pass
        kwargs["trace"] = True
    res = bass_utils.run_bass_kernel_spmd(
        nc, in_maps, list(range(NCORES)), **kwargs
    )
    out = np.empty((B, S, E), np.float32)
    for b in range(B):
        acc = res.results[4 * b]["out"].astype(np.float32).copy()
        for g in range(1, 4):
            acc += res.results[4 * b + g]["out"]
        out[b] = acc + bp[None, :]
    return out, res.exec_time_ns


def kernel(**inputs):
    out, _ = run(inputs, trace=False)
    return out


# revision 17
# speedup vs baseline: 1.0644x; 1.0644x over previous
"""Multi-head causal self-attention (B=2, S=2048, E=1024, H=16, D=64) on 8 TRN2
NeuronCores.

Sharding: core c owns batch b = c//4 and head-group g = c%4 (4 heads each).
Per core, transpose-free layout: QT/KT [d_local=256, S] (d on partitions),
V [S, d_local] (t on partitions), scoresT [t, s] blocks via lhsT=KT-block,
softmax is unnormalized exp (scores ~N(0,1) in f32), sums come free from a
ones-augmented V ([V|1] -> M=65 AV matmul, row 64 = column sums), and the
output projection is row-parallel: each core computes a partial [S, E]; the
host sums the 4 partials per batch and adds bp.

v2 (software pipeline): QKV/V projection chains for s-block i+1 and the
previous block's output-projection units are DRIPPED one per j-iteration into
s-block i's attention loop, so the PE never idles at s-block boundaries (the
baseline went HAM-cold there 4x, ~26us penalty) and ScalarE exp streams
back-to-back from ~12us onward. Attention runs pair-major (head pair p's full
j-loop, then p+1) halving live AV PSUM tiles; PSUM pools: scores 2x[128,1024]
(4 banks), AV 2x[65,512] (2), shared acc 2x[128,512] (2). All PSUM->SBUF
consumers sit on VectorE (bias-adds, out copies, normalize) so the busy
ScalarE FIFO never gates PSUM recycling; softmax recip comes from DVE
reciprocal_approx_fast directly on the AV ones-row (no ACT rsum copies).

Weights/activations cast to bf16 host-side (matmul inputs); accumulation f32
in PSUM; softmax exp in f32.
"""

import numpy as np
import ml_dtypes

import concourse.bass as bass
import concourse.tile as tile
from concourse import bacc, mybir
from concourse import bass_utils

B, S, E, H, D = 2, 2048, 1024, 16, 64
NCORES = 8
HPC = 4                 # heads per core
EL = HPC * D            # 256 local channels
SBW = 512               # s-block width
NSB = S // SBW          # 4
TBW = 128               # t-block width
NTB = S // TBW          # 16
NEB = E // 128          # 8 e-blocks
SCALE = 1.0 / np.sqrt(D)

F32 = mybir.dt.float32
BF16 = mybir.dt.bfloat16

_BUILT = None


def _emit(tc, nc, d):
    Exp = mybir.ActivationFunctionType.Exp

    with (
        tc.tile_pool(name="const", bufs=1) as cst,
        tc.tile_pool(name="big", bufs=1) as big,
        tc.tile_pool(name="ptp", bufs=4) as ptp,
        tc.tile_pool(name="rsp", bufs=4) as rsp,
        tc.tile_pool(name="outp", bufs=4) as outp,
        tc.tile_pool(name="scp", bufs=2, space="PSUM") as scp,
        tc.tile_pool(name="avp", bufs=2, space="PSUM") as avp,
        tc.tile_pool(name="accp", bufs=2, space="PSUM") as accp,
    ):
        # ---- load inputs (order = arrival priority; one big DMA per
        # tensor/e-block -- DMA issue costs ~0.6us each, so few+large) ----
        wq = big.tile([128, NEB * EL], BF16, name="wq", tag="wq")
        nc.sync.dma_start(wq[:], d["wq"][:])
        bq = cst.tile([128, 2], F32, name="bq", tag="bq")
        nc.sync.dma_start(bq[:], d["bq"][:])
        wk = big.tile([128, NEB * EL], BF16, name="wk", tag="wk")
        nc.sync.dma_start(wk[:], d["wk"][:])
        bk = cst.tile([128, 2], F32, name="bk", tag="bk")
        nc.sync.dma_start(bk[:], d["bk"][:])
        tri = cst.tile([128, 128], BF16, name="tri", tag="tri")
        nc.sync.dma_start(tri[:], d["tri"][:])
        wv = big.tile([128, NEB * EL], BF16, name="wv", tag="wv")
        nc.sync.dma_start(wv[:], d["wv"][:])
        bv = cst.tile([128, EL], F32, name="bv", tag="bv")
        nc.sync.dma_start(bv[:], d["bv"][:])
        xt = [big.tile([128, S], BF16, name=f"xt{j}", tag=f"xt{j}")
              for j in range(NEB)]
        for j in range(NEB):
            nc.sync.dma_start(xt[j][:], d["xt"][:, j * S:(j + 1) * S])
        wp = big.tile([128, 2 * E], BF16, name="wp", tag="wp")
        nc.sync.dma_start(wp[:], d["wp"][:])
        onesb = cst.tile([128, 64], BF16, name="onesb", tag="onesb")
        nc.vector.memset(onesb[:], 1.0)


        # V tiles [128, 4*65]: head h at cols 65h..65h+64, ones col at 65h+64
        vt = []
        for j in range(NTB):
            t = big.tile([128, HPC * 65], BF16, name=f"vt{j}", tag=f"vt{j}")
            nc.vector.memset(
                t.rearrange("p (h c) -> p h c", c=65)[:, :, 64:65], 1.0
            )
            vt.append(t)

        qt = [big.tile([128, S], BF16, name=f"qt{k}", tag=f"qt{k}")
              for k in range(2)]
        kt = [big.tile([128, S], BF16, name=f"kt{k}", tag=f"kt{k}")
              for k in range(2)]
        yt = [big.tile([128, S], BF16, name=f"yt{k}", tag=f"yt{k}")
              for k in range(2)]

        # ---- pipeline work units (each ~0.5-2us of PE work) ----
        def qkv_chain(i, dst, wl, bl, dt_i):
            def emit():
                ac = accp.tile([128, SBW], F32, name="qk_ac", tag="acc")
                for j in range(NEB):
                    nc.tensor.matmul(
                        ac[:],
                        wl[:, j * EL + dt_i * 128: j * EL + dt_i * 128 + 128],
                        xt[j][:, i * SBW:(i + 1) * SBW],
                        start=(j == 0),
                        stop=(j == NEB - 1),
                    )
                nc.vector.tensor_scalar_add(
                    dst[dt_i][:, i * SBW:(i + 1) * SBW], ac[:],
                    bl[:, dt_i:dt_i + 1],
                )
            return emit

        def v_chain(j16):
            def emit():
                ac = accp.tile([128, EL], F32, name="v_ac", tag="acc")
                for eb in range(NEB):
                    nc.tensor.matmul(
                        ac[:],
                        xt[eb][:, j16 * TBW:(j16 + 1) * TBW],
                        wv[:, eb * EL:(eb + 1) * EL],
                        start=(eb == 0),
                        stop=(eb == NEB - 1),
                    )
                nc.vector.tensor_add(
                    vt[j16].rearrange("p (h c) -> p h c", c=65)[:, :, 0:64],
                    ac.rearrange("p (h c) -> p h c", c=64),
                    bv.rearrange("p (h c) -> p h c", c=64),
                )
            return emit

        def a_units(i):
            u = []
            for dst, wl, bl in ((qt, wq, bq), (kt, wk, bk)):
                for dt_i in range(2):
                    u.append(qkv_chain(i, dst, wl, bl, dt_i))
            for t4 in range(4):
                u.append(v_chain(4 * i + t4))
            return u

        def proj_unit(r0):
            # full [128, 1024] output row-block: 4 MMs, 1 copy, 1 big DMA
            def emit():
                ot = outp.tile([128, E], F32, name="ot", tag="ot")
                for nb2 in range(2):
                    pr = accp.tile([128, 512], F32, name="pr", tag="acc")
                    for cb in range(2):
                        nc.tensor.matmul(
                            pr[:],
                            yt[cb][:, r0:r0 + 128],
                            wp[:, cb * E + nb2 * 512: cb * E + (nb2 + 1) * 512],
                            start=(cb == 0),
                            stop=(cb == 1),
                        )
                    nc.vector.tensor_copy(
                        ot[:, nb2 * 512:(nb2 + 1) * 512], pr[:]
                    )
                nc.sync.dma_start(d["out"][r0:r0 + 128, :], ot[:])
            return emit

        def c_units(i):
            return [proj_unit(i * SBW + st * 128) for st in range(4)]

        pending = []

        def drip(n=1):
            for _ in range(n):
                if pending:
                    pending.pop(0)()

        # ---- A(0) up front: q/k chains interleaved per e-block so the PE
        # consumes x e-blocks as their DMAs land; then V chains ----
        for dt_i in range(2):
            acq = accp.tile([128, SBW], F32, name="acq", tag="acc")
            ack = accp.tile([128, SBW], F32, name="ack", tag="acc")
            for j in range(NEB):
                for ac, wl in ((acq, wq), (ack, wk)):
                    nc.tensor.matmul(
                        ac[:],
                        wl[:, j * EL + dt_i * 128: j * EL + dt_i * 128 + 128],
                        xt[j][:, 0:SBW],
                        start=(j == 0),
                        stop=(j == NEB - 1),
                    )
            nc.vector.tensor_scalar_add(
                qt[dt_i][:, 0:SBW], acq[:], bq[:, dt_i:dt_i + 1])
            nc.vector.tensor_scalar_add(
                kt[dt_i][:, 0:SBW], ack[:], bk[:, dt_i:dt_i + 1])
        for t4 in range(4):
            v_chain(t4)()

        for i in range(NSB):
            if i + 1 < NSB:
                pending.extend(a_units(i + 1))
            njs = 4 * i + 4
            for p in range(2):
                avs = [avp.tile([65, SBW], F32, name=f"av{hh}", tag="av")
                       for hh in range(2)]

                def av_mms(pt_, w_, j_):
                    for hh in range(2):
                        nc.tensor.matmul(
                            avs[hh][:, w_:SBW],
                            vt[j_][:, 65 * (2 * p + hh): 65 * (2 * p + hh) + 65],
                            pt_[:, hh * SBW + w_:(hh + 1) * SBW],
                            start=(j_ == 0),
                            stop=(j_ == njs - 1),
                        )

                prev = None
                for j in range(njs):
                    w = 128 * (j - 4 * i) if j >= 4 * i else 0
                    cw = SBW - w
                    sc2 = scp.tile([128, 2 * SBW], F32, name="sc2", tag="sc")
                    for hh in range(2):
                        po = 64 * hh
                        nc.tensor.matmul(
                            sc2[:, hh * SBW: hh * SBW + cw],
                            kt[p][po:po + 64, j * TBW:(j + 1) * TBW],
                            qt[p][po:po + 64, i * SBW + w: (i + 1) * SBW],
                            start=True,
                            stop=True,
                        )
                    pt_t = ptp.tile([128, 2 * SBW], BF16, name="ptile",
                                    tag="pt")
                    nc.scalar.activation(
                        pt_t.rearrange("q (g c) -> q g c", c=SBW)[:, :, w:SBW],
                        sc2.rearrange("q (g c) -> q g c", c=SBW)[:, :, 0:cw],
                        Exp,
                    )
                    if j >= 4 * i:  # diagonal: 0/1 triangular mask on PT
                        for hh in range(2):
                            zone = hh * SBW + w
                            nc.vector.tensor_mul(
                                pt_t[:, zone: zone + 128],
                                pt_t[:, zone: zone + 128],
                                tri[:],
                            )
                    if prev is not None:
                        av_mms(*prev)
                    prev = (pt_t, w, j)
                    drip()
                av_mms(*prev)
                # normalize pair p: yt[p][64hh+..] = av[0:64] / av[64]
                # (DVE copy-cast the ones-row to bf16 SBUF, PE-broadcast it
                #  to 64 partitions with a K=1 bf16 matmul, recip on DVE)
                for hh in range(2):
                    rsum = rsp.tile([65, SBW], BF16, name="rsum", tag="rs")
                    nc.vector.tensor_copy(rsum[64:65, :], avs[hh][64:65, :])
                    bc = accp.tile([64, SBW], F32, name="bc", tag="acc")
                    nc.tensor.matmul(
                        bc[:], onesb[64:65, 0:64], rsum[64:65, :],
                        start=True, stop=True,
                    )
                    bcr = rsp.tile([64, SBW], F32, name="bcr", tag="bcs")
                    nc.vector.reciprocal_approx_fast(bcr[:], bc[:])
                    nc.vector.tensor_mul(
                        yt[p][64 * hh: 64 * hh + 64, i * SBW:(i + 1) * SBW],
                        avs[hh][0:64, :],
                        bcr[:],
                    )
            cu = c_units(i)
            if i < NSB - 1:
                pending.extend(cu)
            else:
                for u in cu:
                    u()


def _build():
    global _BUILT
    if _BUILT is not None:
        return _BUILT
    nc = bacc.Bacc("TRN2", target_bir_lowering=False, debug=False,
                   num_devices=NCORES)
    d = {
        "xt": nc.dram_tensor("xt", [128, NEB * S], BF16, kind="ExternalInput").ap(),
        "wq": nc.dram_tensor("wq", [128, NEB * EL], BF16, kind="ExternalInput").ap(),
        "wk": nc.dram_tensor("wk", [128, NEB * EL], BF16, kind="ExternalInput").ap(),
        "wv": nc.dram_tensor("wv", [128, NEB * EL], BF16, kind="ExternalInput").ap(),
        "wp": nc.dram_tensor("wp", [128, 2 * E], BF16, kind="ExternalInput").ap(),
        "bq": nc.dram_tensor("bq", [128, 2], F32, kind="ExternalInput").ap(),
        "bk": nc.dram_tensor("bk", [128, 2], F32, kind="ExternalInput").ap(),
        "bv": nc.dram_tensor("bv", [128, EL], F32, kind="ExternalInput").ap(),
        "tri": nc.dram_tensor("tri", [128, 128], BF16, kind="ExternalInput").ap(),
        "out": nc.dram_tensor("out", [S, E], F32, kind="ExternalOutput").ap(),
    }
    with tile.TileContext(nc) as tc:
        _emit(tc, nc, d)
    nc.compile()
    _BUILT = nc
    return _BUILT


def _blockify(a, pblk):
    """[N*pblk, M] -> [pblk, N*M] with block-column layout."""
    n = a.shape[0] // pblk
    return np.ascontiguousarray(
        a.reshape(n, pblk, a.shape[1]).transpose(1, 0, 2).reshape(pblk, -1)
    )


def _prep_core(c, x, Wq, bq, Wk, bk, Wv, bv, Wp):
    b, g = c // 4, c % 4
    lo = EL * g
    bf = ml_dtypes.bfloat16

    xT = np.ascontiguousarray(x[b].T)                        # [E, S]
    wqT = np.ascontiguousarray(Wq[lo:lo + EL, :].T) * SCALE  # [E, 256]
    wkT = np.ascontiguousarray(Wk[lo:lo + EL, :].T)
    wvT = np.ascontiguousarray(Wv[lo:lo + EL, :].T)
    wpT = np.ascontiguousarray(Wp[:, lo:lo + EL].T)          # [256, E]

    col = np.arange(128, dtype=np.int64)
    tri = np.where(col[None, :] >= np.arange(128)[:, None], 1.0, 0.0)

    return {
        "xt": _blockify(xT, 128).astype(bf),
        "wq": _blockify(wqT, 128).astype(bf),
        "wk": _blockify(wkT, 128).astype(bf),
        "wv": _blockify(wvT, 128).astype(bf),
        "wp": _blockify(wpT, 128).astype(bf),
        "bq": np.ascontiguousarray(
            (bq[lo:lo + EL] * SCALE).reshape(2, 128).T).astype(np.float32),
        "bk": np.ascontiguousarray(
            bk[lo:lo + EL].reshape(2, 128).T).astype(np.float32),
        "bv": np.ascontiguousarray(
            np.broadcast_to(bv[lo:lo + EL], (128, EL))).astype(np.float32),
        "tri": tri.astype(bf),
    }


def run(inputs, trace=False):
    """Run on hardware. Returns (out [B,S,E] f32, exec_time_ns or None)."""
    x = np.asarray(inputs["x"], np.float32)
    Wq = np.asarray(inputs["Wq"], np.float32)
    bq = np.asarray(inputs["bq"], np.float32)
    Wk = np.asarray(inputs["Wk"], np.float32)
    bk = np.asarray(inputs["bk"], np.float32)
    Wv = np.asarray(inputs["Wv"], np.float32)
    bv = np.asarray(inputs["bv"], np.float32)
    Wp = np.asarray(inputs["Wp"], np.float32)
    bp = np.asarray(inputs["bp"], np.float32)

    nc = _build()
    in_maps = [
        _prep_core(c, x, Wq, bq, Wk, bk, Wv, bv, Wp) for c in range(NCORES)
    ]
    kwargs = {}
    if trace:
        try:
            import ntff_shim
            ntff_shim.install()
        except Exception:
            pass
        kwargs["trace"] = True
    res = bass_utils.run_bass_kernel_spmd(
        nc, in_maps, list(range(NCORES)), **kwargs
    )
    out = np.empty((B, S, E), np.float32)
    for b in range(B):
        acc = res.results[4 * b]["out"].astype(np.float32).copy()
        for g in range(1, 4):
            acc += res.results[4 * b + g]["out"]
        out[b] = acc + bp[None, :]
    return out, res.exec_time_ns


def kernel(**inputs):
    out, _ = run(inputs, trace=False)
    return out
